# revision 21
# baseline (speedup 1.0000x reference)
"""GQA attention (B=2,S=2048,DIM=4096,NH=32,NKV=8,HD=128) on 8 TRN2 NeuronCores.

Tensor-parallel over KV groups: core c owns q-heads [4c,4c+4), kv-head c and
wo columns [512c,512c+512). x is replicated (pre-transposed to feature-major,
cast to fp16), each core computes a partial (T,DIM) fp16 output of the wo
matmul; the host sums the 8 partials in fp32.

v2 vs baseline:
- Phase 2: denominator accumulated once per (b,h) into a [4,512] PSUM tile
  via column-selecting stationaries; ONE batched DVE reciprocal per (b,h)
  (split in two halves), normalization (broadcast matmul + multiply + store)
  deferred into the next (b,h)'s instruction stream so the tensor queue never
  waits on the reciprocal (the 3.4us stalls re-throttled the PE to 1.2GHz).
- Causal masks: only the triangular 128x128 diagonal sub-block is added on
  DVE; fully-masked strips are handled by memset of the exp tile + restricting
  the Exp activation's column range.
- K/V loaded once per batch (shared across the 4 heads).
- Startup DMAs chunked so the first matmul starts ~10us in, not 43us.
- Phase 3 writes fp16 partials (half the DMA) and orders loops jo-major.
"""

import math

import ml_dtypes
import numpy as np

import concourse.bass as bass
import concourse.mybir as mybir
import concourse.tile as tile
from concourse import bacc
from concourse.bass_utils import run_bass_kernel_spmd

B, S, DIM = 2, 2048, 4096
NH, NKV, HD = 32, 8, 128
T = B * S
N_CORES = 8
QPC = (NH // N_CORES) * HD          # 512 q-dims per core
NHEAD = NH // N_CORES               # 4 q heads per core
P = 128
F32 = mybir.dt.float32
F16 = mybir.dt.float16
BF16 = mybir.dt.bfloat16
SCALE = 1.0 / math.sqrt(HD)

TT1 = 512                           # phase-1 token tile
NT1 = T // TT1                      # 8
KO = DIM // P                       # 32 contraction tiles
TT2 = 512                           # phase-2 t tile
NT2 = S // TT2                      # 4 t-tiles per (b,h)
TT3 = 128                           # phase-3 token tile
IT3 = 512                           # phase-3 output-column tile


def build_kernel() -> bass.Bass:
    nc = bacc.Bacc()

    xT = nc.declare_dram_parameter("xT", [DIM, T], F16, isOutput=False)
    wqT = nc.declare_dram_parameter("wqT", [DIM, QPC], F16, isOutput=False)
    wkT = nc.declare_dram_parameter("wkT", [DIM, HD], F16, isOutput=False)
    wvT = nc.declare_dram_parameter("wvT", [DIM, HD], F16, isOutput=False)
    woT = nc.declare_dram_parameter("woT", [QPC, DIM], F16, isOutput=False)
    ropeC = nc.declare_dram_parameter("ropeC", [P, S], F16, isOutput=False)
    ropeS = nc.declare_dram_parameter("ropeS", [P, S], F16, isOutput=False)
    permM = nc.declare_dram_parameter("permM", [P, P], F16, isOutput=False)
    identM = nc.declare_dram_parameter("identM", [P, P], F16, isOutput=False)
    colC4M = nc.declare_dram_parameter("colC4M", [P, 4 * NT2], BF16,
                                       isOutput=False)
    triM = nc.declare_dram_parameter("triM", [P, P], F32, isOutput=False)
    biasM = nc.declare_dram_parameter("biasM", [P, 1], F32, isOutput=False)
    out = nc.declare_dram_parameter("out_part", [T, DIM], F16, isOutput=True)

    qT_d = [nc.dram_tensor(f"qT_d{b}", [QPC, S], F16) for b in range(B)]
    kT_d = [nc.dram_tensor(f"kT_d{b}", [HD, S], F16) for b in range(B)]
    v_d = [nc.dram_tensor(f"v_d{b}", [S, HD], BF16) for b in range(B)]
    ao_d = [nc.dram_tensor(f"ao_d{b}", [QPC, S], F16) for b in range(B)]

    with tile.TileContext(nc) as tc:
        # wo preloaded at kernel start so phase 3 never waits on its 4MB DMA
        wpool3 = tc.alloc_tile_pool(name="p3w", bufs=1)
        # phase-2 K/V/Q pools created early so batch-0 tiles can prefetch
        # while phase 1 is still running
        kvpool = tc.alloc_tile_pool(name="p2kv", bufs=2)
        qpool = tc.alloc_tile_pool(name="p2q", bufs=2)
        prefetch = {}
        # ---------------- Phase 1: QKV projections + RoPE + V transpose ----
        with (
            tc.tile_pool(name="p1w", bufs=1) as wpool,
            tc.tile_pool(name="p1x", bufs=2) as xpool,
            tc.tile_pool(name="p1s", bufs=3) as spool,
            tc.tile_pool(name="p1ps", bufs=6, space="PSUM") as pspool,
            tc.tile_pool(name="p1ps2", bufs=2, space="PSUM") as pspool2,
        ):
            # chunked weight loads so the first matmuls can start early
            wq_sb = wpool.tile([P, KO, QPC], F16)
            wqr = wqT.rearrange("(ko p) m -> p ko m", p=P)
            nc.sync.dma_start(wq_sb[:, 0:8, :], wqr[:, 0:8, :])
            wk_sb = wpool.tile([P, KO, HD], F16)
            nc.sync.dma_start(wk_sb[:], wkT.rearrange("(ko p) m -> p ko m", p=P))
            wv_sb = wpool.tile([P, KO, HD], F16)
            nc.sync.dma_start(wv_sb[:], wvT.rearrange("(ko p) m -> p ko m", p=P))
            for c in range(1, 4):
                nc.sync.dma_start(wq_sb[:, 8 * c:8 * c + 8, :],
                                  wqr[:, 8 * c:8 * c + 8, :])
            perm_sb = wpool.tile([P, P], F16)
            nc.sync.dma_start(perm_sb[:], permM[:])
            ident_sb = wpool.tile([P, P], F16)
            nc.sync.dma_start(ident_sb[:], identM[:])
            ropeC_sb = wpool.tile([P, S], F16)
            nc.sync.dma_start(ropeC_sb[:], ropeC[:])
            ropeS_sb = wpool.tile([P, S], F16)
            nc.sync.dma_start(ropeS_sb[:], ropeS[:])
            wo_sb = wpool3.tile([P, QPC // P, DIM], F16)
            nc.sync.dma_start(wo_sb[:], woT.rearrange("(jo p) i -> p jo i", p=P))

            # One accumulator (=1 PSUM bank) per output slice; RoPE/transpose
            # for slice a is issued after slice a+1's matmuls so the tensor
            # queue never waits on the PSUM->SBUF copies.
            pending1 = []

            def rope_tail(b, lt0, j, acc):
                raw = spool.tile([P, TT1], F16, tag="raw")
                nc.any.tensor_copy(raw[:], acc[:])
                pperm = pspool2.tile([P, TT1], F32, tag="scr2")
                nc.tensor.matmul(pperm[:], perm_sb[:], raw[:],
                                 start=True, stop=True)
                swp = spool.tile([P, TT1], F16, tag="swp")
                nc.any.tensor_copy(swp[:], pperm[:])
                roped = spool.tile([P, TT1], F16, tag="roped")
                nc.vector.tensor_mul(roped[:], raw[:],
                                     ropeC_sb[:, lt0:lt0 + TT1])
                swapped = spool.tile([P, TT1], F16, tag="swapped")
                nc.vector.tensor_mul(swapped[:], swp[:],
                                     ropeS_sb[:, lt0:lt0 + TT1])
                nc.vector.tensor_add(roped[:], roped[:], swapped[:])
                if j < 4:
                    nc.sync.dma_start(
                        qT_d[b][j * P:(j + 1) * P, lt0:lt0 + TT1], roped[:]
                    )
                else:
                    nc.sync.dma_start(kT_d[b][:, lt0:lt0 + TT1], roped[:])

            def v_tail(b, lt0, acc):
                vraw = spool.tile([P, TT1], F16, tag="vraw")
                nc.any.tensor_copy(vraw[:], acc[:])
                for j in range(TT1 // P):
                    pt_full = pspool2.tile([P, TT1], F16, tag="scr2", name="pt")
                    pt = pt_full[:, :P]
                    nc.tensor.transpose(pt[:], vraw[:, j * P:(j + 1) * P],
                                        ident_sb[:])
                    vsd = spool.tile([P, P], BF16, tag="vsd")
                    nc.any.tensor_copy(vsd[:], pt[:])
                    nc.sync.dma_start(
                        v_d[b][lt0 + j * P:lt0 + (j + 1) * P, :], vsd[:]
                    )

            def w_of(a):
                if a < 4:
                    return wq_sb, a * P
                return (wk_sb, 0) if a == 4 else (wv_sb, 0)

            for ti in range(NT1):
                b = ti // (NT1 // B)
                lt0 = (ti % (NT1 // B)) * TT1   # within-batch column offset
                xfull = xpool.tile([P, KO, TT1], F16, tag="xfull")
                xr = xT[:, ti * TT1:(ti + 1) * TT1].rearrange(
                    "(ko p) t -> p ko t", p=P)
                if ti == 0:     # fine chunks: first matmul starts ~4us in
                    for qtr in range(4):
                        nc.scalar.dma_start(xfull[:, 8 * qtr:8 * qtr + 8, :],
                                            xr[:, 8 * qtr:8 * qtr + 8, :])
                else:
                    nc.scalar.dma_start(xfull[:, 0:16, :], xr[:, 0:16, :])
                    nc.scalar.dma_start(xfull[:, 16:32, :], xr[:, 16:32, :])
                if ti == 0:
                    # ko-outer so compute starts after the first x quarter
                    accs = [pspool.tile([P, TT1], F32, tag="acc",
                                        name=f"acc0_{a}") for a in range(6)]
                    for ko in range(KO):
                        for a in range(6):
                            w_sb, c0 = w_of(a)
                            nc.tensor.matmul(
                                accs[a][:], w_sb[:, ko, c0:c0 + P],
                                xfull[:, ko, :],
                                start=(ko == 0), stop=(ko == KO - 1),
                            )
                    for a in range(5):
                        pending1.append(
                            lambda b=b, lt0=lt0, j=a, acc=accs[a]:
                                rope_tail(b, lt0, j, acc))
                    pending1.append(
                        lambda b=b, lt0=lt0, acc=accs[5]: v_tail(b, lt0, acc))
                    continue
                for a in range(6):
                    acc = pspool.tile([P, TT1], F32, tag="acc")
                    w_sb, c0 = w_of(a)
                    for ko in range(KO):
                        nc.tensor.matmul(
                            acc[:], w_sb[:, ko, c0:c0 + P], xfull[:, ko, :],
                            start=(ko == 0), stop=(ko == KO - 1),
                        )
                    for fn in pending1[:2]:
                        fn()
                    del pending1[:2]
                    if a < 5:
                        pending1.append(
                            lambda b=b, lt0=lt0, j=a, acc=acc:
                                rope_tail(b, lt0, j, acc))
                    else:
                        pending1.append(
                            lambda b=b, lt0=lt0, acc=acc: v_tail(b, lt0, acc))
                if ti == 3:
                    # all b0 stores must be traced before the loads below,
                    # or the dep tracker misses the RAW hazard
                    for fn in pending1:
                        fn()
                    pending1.clear()
                    # batch-0 K/V/Q prefetch: issued on sync, waits only on
                    # b0's own stores, streams in while phase 1 does b1
                    kh0 = kvpool.tile([P, S], F16, tag="kh", name="kh_pf")
                    nc.sync.dma_start(kh0[:], kT_d[0][:, :])
                    vh0 = kvpool.tile([P, S // P, HD], BF16, tag="vh",
                                      name="vh_pf")
                    nc.sync.dma_start(
                        vh0[:],
                        v_d[0][:, :].rearrange("(so p) d -> p so d", p=P))
                    qh_pf = []
                    for hh in range(2):
                        qht = qpool.tile([P, S], F16, tag="qh",
                                         name=f"qh_pf{hh}")
                        nc.sync.dma_start(
                            qht[:], qT_d[0][hh * P:(hh + 1) * P, :])
                        qh_pf.append(qht)
                    prefetch.update(kh0=kh0, vh0=vh0, qh=qh_pf)
            for fn in pending1:
                fn()
            pending1.clear()

        # ---------------- Phase 2: causal attention per (b, h) -------------
        with (
            tc.tile_pool(name="p2c", bufs=1) as cpool,
            tc.tile_pool(name="p2e", bufs=5) as epool,
            tc.tile_pool(name="p2o", bufs=2) as opool,
            tc.tile_pool(name="p2ps_sc", bufs=5, space="PSUM") as ps_sc,
            tc.tile_pool(name="p2ps_o", bufs=1, space="PSUM") as ps_o,
            tc.tile_pool(name="p2ps_den", bufs=2, space="PSUM") as ps_den,
        ):
            colC4_sb = cpool.tile([P, 4 * NT2], BF16)
            nc.sync.dma_start(colC4_sb[:], colC4M[:])
            tri_sb = cpool.tile([P, P], F32)
            nc.sync.dma_start(tri_sb[:], triM[:])
            bias_sb = cpool.tile([P, 1], F32)
            nc.sync.dma_start(bias_sb[:], biasM[:])
            ones32 = cpool.tile([1, P], F32)
            nc.gpsimd.memset(ones32[:], 1.0)

            pending = []            # deferred work: (stage, closure)

            def flush(stage):
                keep = []
                for st, fn in pending:
                    if st == stage:
                        fn()
                    else:
                        keep.append((st, fn))
                pending[:] = keep

            for b in range(B):
                if b == 0:
                    kh, vh = prefetch["kh0"], prefetch["vh0"]
                else:
                    kh = kvpool.tile([P, S], F16, tag="kh")
                    nc.sync.dma_start(kh[:], kT_d[b][:, :])
                    vh = kvpool.tile([P, S // P, HD], BF16, tag="vh")
                    nc.sync.dma_start(
                        vh[:],
                        v_d[b][:, :].rearrange("(so p) d -> p so d", p=P)
                    )
                for h in range(NHEAD):
                    if b == 0 and h < 2:
                        qh = prefetch["qh"][h]
                    else:
                        qh = qpool.tile([P, S], F16, tag="qh")
                        nc.gpsimd.dma_start(qh[:],
                                            qT_d[b][h * P:(h + 1) * P, :])
                    psden = ps_den.tile([4, TT2], F32, tag="psden")
                    aouns = []

                    for t_idx in range(NT2):
                        if t_idx == 1:
                            flush(0)    # prev (b,h): reciprocal halves
                        t0 = t_idx * TT2
                        n_s = 4 * (t_idx + 1)   # visible 128-wide s-tiles
                        psO = ps_o.tile([P, TT2], F32, tag="psO")
                        for st in range(n_s):
                            if st == 4:
                                if t_idx == 2:
                                    flush(1)    # prev: normalize t0,t1
                                elif t_idx == 3:
                                    flush(2)    # prev: normalize t2,t3
                            pssc = ps_sc.tile([P, TT2], F32, tag="pssc")
                            nc.tensor.matmul(
                                pssc[:], kh[:, st * P:(st + 1) * P],
                                qh[:, t0:t0 + TT2], start=True, stop=True,
                            )
                            esb = epool.tile([P, TT2], BF16, tag="esb")
                            dg = st - 4 * t_idx
                            if dg < 0:          # fully visible s-tile
                                nc.scalar.activation(
                                    esb[:], pssc[:],
                                    mybir.ActivationFunctionType.Exp,
                                    scale=SCALE, bias=bias_sb[:, 0:1],
                                )
                            else:               # diagonal s-tile
                                c0 = P * dg
                                nc.vector.tensor_add(
                                    pssc[:, c0:c0 + P],
                                    pssc[:, c0:c0 + P], tri_sb[:],
                                )
                                if dg > 0:
                                    nc.vector.memset(esb[:, 0:c0], 0.0)
                                nc.scalar.activation(
                                    esb[:, c0:TT2], pssc[:, c0:TT2],
                                    mybir.ActivationFunctionType.Exp,
                                    scale=SCALE, bias=bias_sb[:, 0:1],
                                )
                            z0 = P * dg if dg > 0 else 0
                            nc.tensor.matmul(
                                psO[:, z0:TT2], vh[:, st, :], esb[:, z0:TT2],
                                start=(st == 0), stop=(st == n_s - 1),
                            )
                            nc.tensor.matmul(
                                psden[:, z0:TT2],
                                colC4_sb[:, 4 * t_idx:4 * t_idx + 4],
                                esb[:, z0:TT2],
                                start=(t_idx == 0 and st == 0),
                                stop=(t_idx == NT2 - 1 and st == n_s - 1),
                            )
                        # /256 keeps the unnormalized sum inside f16 range;
                        # recip carries the matching 256/sum(e) factor.
                        aoun = opool.tile([P, TT2], F16, tag="aoun", bufs=8)
                        nc.vector.tensor_scalar_mul(aoun[:], psO[:], 1.0 / 256.0)
                        aouns.append(aoun)

                    # deferred tail for this (b,h): recip + normalize + store
                    recip4 = opool.tile([4, TT2], F32, tag="recip4")
                    rflat = opool.tile([1, 4, TT2], F32, tag="rflat",
                                       bufs=1)

                    def mk_recip(psden=psden, recip4=recip4, rflat=rflat):
                        def go():
                            nc.vector.reciprocal(recip4[:, 0:TT2 // 2],
                                                 psden[:, 0:TT2 // 2])
                            nc.vector.reciprocal(recip4[:, TT2 // 2:],
                                                 psden[:, TT2 // 2:])
                            nc.gpsimd.dma_start(rflat[:], recip4[:])
                        return go

                    def mk_norm(t_idx, b=b, h=h, rflat=rflat, aouns=aouns):
                        def go():
                            psbc = ps_sc.tile([P, TT2], F32, tag="pssc",
                                              name="psbc")
                            nc.tensor.matmul(
                                psbc[:], ones32[:],
                                rflat[:, t_idx, :], start=True, stop=True,
                            )
                            osb = opool.tile([P, TT2], F16, tag="osb", bufs=3)
                            nc.vector.tensor_mul(osb[:], aouns[t_idx][:],
                                                 psbc[:])
                            nc.sync.dma_start(
                                ao_d[b][h * P:(h + 1) * P,
                                        t_idx * TT2:(t_idx + 1) * TT2],
                                osb[:],
                            )
                        return go

                    pending.append((0, mk_recip()))
                    pending.append((1, mk_norm(0)))
                    pending.append((1, mk_norm(1)))
                    pending.append((2, mk_norm(2)))
                    pending.append((2, mk_norm(3)))

            for st, fn in pending:      # last (b,h)'s tail
                fn()
            pending.clear()

        qpool.release()
        kvpool.release()
        # ---------------- Phase 3: wo partial projection -------------------
        with (
            tc.tile_pool(name="p3a", bufs=3) as apool,
            tc.tile_pool(name="p3o", bufs=4) as opool3,
            tc.tile_pool(name="p3ps", bufs=4, space="PSUM") as pspool3,
        ):
            for ti in range(T // TT3):
                b = ti // (S // TT3)
                lt0 = (ti % (S // TT3)) * TT3
                t0 = ti * TT3
                ao_sb = apool.tile([P, QPC // P, TT3], F16, tag="ao")
                nc.gpsimd.dma_start(
                    ao_sb[:],
                    ao_d[b][:, lt0:lt0 + TT3].rearrange(
                        "(jo p) t -> p jo t", p=P),
                )
                for half in range(2):
                    psws = []
                    for ii_l in range(4):
                        psw = pspool3.tile([P, IT3], F32, tag="psw",
                                           name=f"psw{ii_l}")
                        psws.append(psw)
                    for jo in range(QPC // P):
                        for ii_l in range(4):
                            ii = half * 4 + ii_l
                            nc.tensor.matmul(
                                psws[ii_l][:], ao_sb[:, jo, :],
                                wo_sb[:, jo, ii * IT3:(ii + 1) * IT3],
                                start=(jo == 0), stop=(jo == QPC // P - 1),
                            )
                    for ii_l in range(4):
                        ii = half * 4 + ii_l
                        ow = opool3.tile([P, IT3], F16, tag="ow")
                        if ii_l % 2 == 0:
                            nc.vector.tensor_copy(ow[:], psws[ii_l][:])
                        else:
                            nc.scalar.copy(ow[:], psws[ii_l][:])
                        nc.sync.dma_start(
                            out[t0:t0 + TT3, ii * IT3:(ii + 1) * IT3], ow[:]
                        )
        wpool3.release()

    return nc


_NC_CACHE = {}


def _host_inputs(x, wq, wk, wv, wo, freqs_cos, freqs_sin):
    f16 = np.float16
    xT = np.ascontiguousarray(x.reshape(T, DIM).T, dtype=f16)
    C = np.ascontiguousarray(np.repeat(freqs_cos.T, 2, axis=0), dtype=f16)
    sign = np.where(np.arange(HD) % 2 == 0, -1.0, 1.0)[:, None].astype(np.float32)
    Sp = np.ascontiguousarray(np.repeat(freqs_sin.T, 2, axis=0) * sign,
                              dtype=f16)
    perm = np.zeros((P, P), f16)
    idx = np.arange(0, P, 2)
    perm[idx, idx + 1] = 1.0
    perm[idx + 1, idx] = 1.0
    ident = np.eye(P, dtype=f16)
    colC4 = np.zeros((P, 4 * NT2), np.float32)
    for t in range(NT2):
        colC4[:, 4 * t + t] = 1.0 / 256.0
    colC4 = colC4.astype(ml_dtypes.bfloat16)
    tri = np.where(np.arange(P)[None, :] >= np.arange(P)[:, None],
                   0.0, -1e9).astype(np.float32)
    biasv = np.full((P, 1), -4.0, np.float32)

    in_maps = []
    for c in range(N_CORES):
        in_maps.append({
            "xT": xT,
            "wqT": np.ascontiguousarray(wq[c * QPC:(c + 1) * QPC, :].T, dtype=f16),
            "wkT": np.ascontiguousarray(wk[c * HD:(c + 1) * HD, :].T, dtype=f16),
            "wvT": np.ascontiguousarray(wv[c * HD:(c + 1) * HD, :].T, dtype=f16),
            "woT": np.ascontiguousarray(wo[:, c * QPC:(c + 1) * QPC].T, dtype=f16),
            "ropeC": C,
            "ropeS": Sp,
            "permM": perm,
            "identM": ident,
            "colC4M": colC4,
            "triM": tri,
            "biasM": biasv,
        })
    return in_maps


def kernel(x, wq, wk, wv, wo, freqs_cos, freqs_sin, start_pos, _trace=False):
    x = np.asarray(x, np.float32)
    in_maps = _host_inputs(
        x, np.asarray(wq, np.float32), np.asarray(wk, np.float32),
        np.asarray(wv, np.float32), np.asarray(wo, np.float32),
        np.asarray(freqs_cos, np.float32), np.asarray(freqs_sin, np.float32),
    )
    if "nc" not in _NC_CACHE:
        nc = build_kernel()
        nc.compile()
        _NC_CACHE["nc"] = nc
    res = run_bass_kernel_spmd(
        _NC_CACHE["nc"], in_maps, list(range(N_CORES)), trace=_trace
    )
    acc = res.results[0]["out_part"].astype(np.float32)
    for c in range(1, N_CORES):
        acc += res.results[c]["out_part"].astype(np.float32)
    out = acc.reshape(B, S, DIM)
    if _trace:
        return out, res
    return out


# revision 22
# speedup vs baseline: 1.0051x; 1.0051x over previous
"""GQA attention (B=2,S=2048,DIM=4096,NH=32,NKV=8,HD=128) on 8 TRN2 NeuronCores.

Tensor-parallel over KV groups: core c owns q-heads [4c,4c+4), kv-head c and
wo columns [512c,512c+512). x is replicated (pre-transposed to feature-major,
cast to fp16), each core computes a partial (T,DIM) fp16 output of the wo
matmul; the host sums the 8 partials in fp32.

v2 vs baseline:
- Phase 2: denominator accumulated once per (b,h) into a [4,512] PSUM tile
  via column-selecting stationaries; ONE batched DVE reciprocal per (b,h)
  (split in two halves), normalization (broadcast matmul + multiply + store)
  deferred into the next (b,h)'s instruction stream so the tensor queue never
  waits on the reciprocal (the 3.4us stalls re-throttled the PE to 1.2GHz).
- Causal masks: only the triangular 128x128 diagonal sub-block is added on
  DVE; fully-masked strips are handled by memset of the exp tile + restricting
  the Exp activation's column range.
- K/V loaded once per batch (shared across the 4 heads).
- Startup DMAs chunked so the first matmul starts ~10us in, not 43us.
- Phase 3 writes fp16 partials (half the DMA) and orders loops jo-major.
"""

import math

import ml_dtypes
import numpy as np

import concourse.bass as bass
import concourse.mybir as mybir
import concourse.tile as tile
from concourse import bacc
from concourse.bass_utils import run_bass_kernel_spmd

B, S, DIM = 2, 2048, 4096
NH, NKV, HD = 32, 8, 128
T = B * S
N_CORES = 8
QPC = (NH // N_CORES) * HD          # 512 q-dims per core
NHEAD = NH // N_CORES               # 4 q heads per core
P = 128
F32 = mybir.dt.float32
F16 = mybir.dt.float16
BF16 = mybir.dt.bfloat16
SCALE = 1.0 / math.sqrt(HD)

TT1 = 512                           # phase-1 token tile
NT1 = T // TT1                      # 8
KO = DIM // P                       # 32 contraction tiles
TT2 = 512                           # phase-2 t tile
NT2 = S // TT2                      # 4 t-tiles per (b,h)
TT3 = 128                           # phase-3 token tile
IT3 = 512                           # phase-3 output-column tile


def build_kernel() -> bass.Bass:
    nc = bacc.Bacc()

    xT = nc.declare_dram_parameter("xT", [DIM, T], F16, isOutput=False)
    wqT = nc.declare_dram_parameter("wqT", [DIM, QPC], F16, isOutput=False)
    wkT = nc.declare_dram_parameter("wkT", [DIM, HD], F16, isOutput=False)
    wvT = nc.declare_dram_parameter("wvT", [DIM, HD], F16, isOutput=False)
    woT = nc.declare_dram_parameter("woT", [QPC, DIM], F16, isOutput=False)
    ropeC = nc.declare_dram_parameter("ropeC", [P, S], F16, isOutput=False)
    ropeS = nc.declare_dram_parameter("ropeS", [P, S], F16, isOutput=False)
    permM = nc.declare_dram_parameter("permM", [P, P], F16, isOutput=False)
    identM = nc.declare_dram_parameter("identM", [P, P], F16, isOutput=False)
    colC4M = nc.declare_dram_parameter("colC4M", [P, 4 * NT2], BF16,
                                       isOutput=False)
    triM = nc.declare_dram_parameter("triM", [P, P], F32, isOutput=False)
    biasM = nc.declare_dram_parameter("biasM", [P, 1], F32, isOutput=False)
    out = nc.declare_dram_parameter("out_part", [T, DIM], F16, isOutput=True)

    qT_d = [nc.dram_tensor(f"qT_d{b}", [QPC, S], F16) for b in range(B)]
    kT_d = [nc.dram_tensor(f"kT_d{b}", [HD, S], F16) for b in range(B)]
    v_d = [nc.dram_tensor(f"v_d{b}", [S, HD], BF16) for b in range(B)]
    ao_d = [nc.dram_tensor(f"ao_d{b}", [QPC, S], F16) for b in range(B)]

    with tile.TileContext(nc) as tc:
        # wo preloaded at kernel start so phase 3 never waits on its 4MB DMA
        wpool3 = tc.alloc_tile_pool(name="p3w", bufs=1)
        # phase-2 K/V/Q pools created early so batch-0 tiles can prefetch
        # while phase 1 is still running
        kvpool = tc.alloc_tile_pool(name="p2kv", bufs=2)
        qpool = tc.alloc_tile_pool(name="p2q", bufs=2)
        prefetch = {}
        # ---------------- Phase 1: QKV projections + RoPE + V transpose ----
        with (
            tc.tile_pool(name="p1w", bufs=1) as wpool,
            tc.tile_pool(name="p1x", bufs=2) as xpool,
            tc.tile_pool(name="p1s", bufs=3) as spool,
            tc.tile_pool(name="p1ps", bufs=6, space="PSUM") as pspool,
            tc.tile_pool(name="p1ps2", bufs=2, space="PSUM") as pspool2,
        ):
            # chunked weight loads, split across the sync and gpsimd queues
            # so the first chunk-0 ko sweep never outruns the weight stream
            wq_sb = wpool.tile([P, KO, QPC], F16)
            wqr = wqT.rearrange("(ko p) m -> p ko m", p=P)
            nc.sync.dma_start(wq_sb[:, 0:8, :], wqr[:, 0:8, :])
            wk_sb = wpool.tile([P, KO, HD], F16)
            nc.gpsimd.dma_start(wk_sb[:],
                                wkT.rearrange("(ko p) m -> p ko m", p=P))
            wv_sb = wpool.tile([P, KO, HD], F16)
            nc.gpsimd.dma_start(wv_sb[:],
                                wvT.rearrange("(ko p) m -> p ko m", p=P))
            for c in range(1, 4):
                eng = nc.sync if c % 2 == 1 else nc.gpsimd
                eng.dma_start(wq_sb[:, 8 * c:8 * c + 8, :],
                              wqr[:, 8 * c:8 * c + 8, :])
            perm_sb = wpool.tile([P, P], F16)
            nc.sync.dma_start(perm_sb[:], permM[:])
            ident_sb = wpool.tile([P, P], F16)
            nc.sync.dma_start(ident_sb[:], identM[:])
            ropeC_sb = wpool.tile([P, S], F16)
            nc.sync.dma_start(ropeC_sb[:], ropeC[:])
            ropeS_sb = wpool.tile([P, S], F16)
            nc.sync.dma_start(ropeS_sb[:], ropeS[:])
            wo_sb = wpool3.tile([P, QPC // P, DIM], F16)
            nc.sync.dma_start(wo_sb[:], woT.rearrange("(jo p) i -> p jo i", p=P))

            # One accumulator (=1 PSUM bank) per output slice; RoPE/transpose
            # for slice a is issued after slice a+1's matmuls so the tensor
            # queue never waits on the PSUM->SBUF copies.
            pending1 = []

            def rope_tail(b, lt0, j, acc):
                raw = spool.tile([P, TT1], F16, tag="raw")
                nc.any.tensor_copy(raw[:], acc[:])
                pperm = pspool2.tile([P, TT1], F32, tag="scr2")
                nc.tensor.matmul(pperm[:], perm_sb[:], raw[:],
                                 start=True, stop=True)
                swp = spool.tile([P, TT1], F16, tag="swp")
                nc.any.tensor_copy(swp[:], pperm[:])
                roped = spool.tile([P, TT1], F16, tag="roped")
                nc.vector.tensor_mul(roped[:], raw[:],
                                     ropeC_sb[:, lt0:lt0 + TT1])
                swapped = spool.tile([P, TT1], F16, tag="swapped")
                nc.vector.tensor_mul(swapped[:], swp[:],
                                     ropeS_sb[:, lt0:lt0 + TT1])
                nc.vector.tensor_add(roped[:], roped[:], swapped[:])
                if j < 4:
                    nc.sync.dma_start(
                        qT_d[b][j * P:(j + 1) * P, lt0:lt0 + TT1], roped[:]
                    )
                else:
                    nc.sync.dma_start(kT_d[b][:, lt0:lt0 + TT1], roped[:])

            def v_tail(b, lt0, acc):
                vraw = spool.tile([P, TT1], F16, tag="vraw")
                nc.any.tensor_copy(vraw[:], acc[:])
                for j in range(TT1 // P):
                    pt_full = pspool2.tile([P, TT1], F16, tag="scr2", name="pt")
                    pt = pt_full[:, :P]
                    nc.tensor.transpose(pt[:], vraw[:, j * P:(j + 1) * P],
                                        ident_sb[:])
                    vsd = spool.tile([P, P], BF16, tag="vsd")
                    nc.any.tensor_copy(vsd[:], pt[:])
                    nc.sync.dma_start(
                        v_d[b][lt0 + j * P:lt0 + (j + 1) * P, :], vsd[:]
                    )

            def w_of(a):
                if a < 4:
                    return wq_sb, a * P
                return (wk_sb, 0) if a == 4 else (wv_sb, 0)

            for ti in range(NT1):
                b = ti // (NT1 // B)
                lt0 = (ti % (NT1 // B)) * TT1   # within-batch column offset
                xfull = xpool.tile([P, KO, TT1], F16, tag="xfull")
                xr = xT[:, ti * TT1:(ti + 1) * TT1].rearrange(
                    "(ko p) t -> p ko t", p=P)
                if ti == 0:     # fine chunks: first matmul starts ~4us in
                    for qtr in range(4):
                        nc.scalar.dma_start(xfull[:, 8 * qtr:8 * qtr + 8, :],
                                            xr[:, 8 * qtr:8 * qtr + 8, :])
                else:
                    nc.scalar.dma_start(xfull[:, 0:16, :], xr[:, 0:16, :])
                    nc.scalar.dma_start(xfull[:, 16:32, :], xr[:, 16:32, :])
                if ti == 0:
                    # ko-outer so compute starts after the first x quarter
                    accs = [pspool.tile([P, TT1], F32, tag="acc",
                                        name=f"acc0_{a}") for a in range(6)]
                    for ko in range(KO):
                        for a in range(6):
                            w_sb, c0 = w_of(a)
                            nc.tensor.matmul(
                                accs[a][:], w_sb[:, ko, c0:c0 + P],
                                xfull[:, ko, :],
                                start=(ko == 0), stop=(ko == KO - 1),
                            )
                    for a in range(5):
                        pending1.append(
                            lambda b=b, lt0=lt0, j=a, acc=accs[a]:
                                rope_tail(b, lt0, j, acc))
                    pending1.append(
                        lambda b=b, lt0=lt0, acc=accs[5]: v_tail(b, lt0, acc))
                    continue
                for a in range(6):
                    acc = pspool.tile([P, TT1], F32, tag="acc")
                    w_sb, c0 = w_of(a)
                    for ko in range(KO):
                        nc.tensor.matmul(
                            acc[:], w_sb[:, ko, c0:c0 + P], xfull[:, ko, :],
                            start=(ko == 0), stop=(ko == KO - 1),
                        )
                    for fn in pending1[:2]:
                        fn()
                    del pending1[:2]
                    if a < 5:
                        pending1.append(
                            lambda b=b, lt0=lt0, j=a, acc=acc:
                                rope_tail(b, lt0, j, acc))
                    else:
                        pending1.append(
                            lambda b=b, lt0=lt0, acc=acc: v_tail(b, lt0, acc))
                if ti == 3:
                    # all b0 stores must be traced before the loads below,
                    # or the dep tracker misses the RAW hazard
                    for fn in pending1:
                        fn()
                    pending1.clear()
                    # batch-0 K/V/Q prefetch: issued on sync, waits only on
                    # b0's own stores, streams in while phase 1 does b1
                    kh0 = kvpool.tile([P, S], F16, tag="kh", name="kh_pf")
                    nc.sync.dma_start(kh0[:], kT_d[0][:, :])
                    vh0 = kvpool.tile([P, S // P, HD], BF16, tag="vh",
                                      name="vh_pf")
                    nc.sync.dma_start(
                        vh0[:],
                        v_d[0][:, :].rearrange("(so p) d -> p so d", p=P))
                    qh_pf = []
                    for hh in range(2):
                        qht = qpool.tile([P, S], F16, tag="qh",
                                         name=f"qh_pf{hh}")
                        nc.sync.dma_start(
                            qht[:], qT_d[0][hh * P:(hh + 1) * P, :])
                        qh_pf.append(qht)
                    prefetch.update(kh0=kh0, vh0=vh0, qh=qh_pf)
            for fn in pending1:
                fn()
            pending1.clear()

        # ---------------- Phase 2: causal attention per (b, h) -------------
        with (
            tc.tile_pool(name="p2c", bufs=1) as cpool,
            tc.tile_pool(name="p2e", bufs=5) as epool,
            tc.tile_pool(name="p2o", bufs=2) as opool,
            tc.tile_pool(name="p2ps_sc", bufs=5, space="PSUM") as ps_sc,
            tc.tile_pool(name="p2ps_o", bufs=1, space="PSUM") as ps_o,
            tc.tile_pool(name="p2ps_den", bufs=2, space="PSUM") as ps_den,
        ):
            colC4_sb = cpool.tile([P, 4 * NT2], BF16)
            nc.sync.dma_start(colC4_sb[:], colC4M[:])
            tri_sb = cpool.tile([P, P], F32)
            nc.sync.dma_start(tri_sb[:], triM[:])
            bias_sb = cpool.tile([P, 1], F32)
            nc.sync.dma_start(bias_sb[:], biasM[:])
            ones32 = cpool.tile([1, P], F32)
            nc.gpsimd.memset(ones32[:], 1.0)

            pending = []            # deferred work: (stage, closure)

            def flush(stage):
                keep = []
                for st, fn in pending:
                    if st == stage:
                        fn()
                    else:
                        keep.append((st, fn))
                pending[:] = keep

            for b in range(B):
                if b == 0:
                    kh, vh = prefetch["kh0"], prefetch["vh0"]
                else:
                    kh = kvpool.tile([P, S], F16, tag="kh")
                    nc.sync.dma_start(kh[:], kT_d[b][:, :])
                    vh = kvpool.tile([P, S // P, HD], BF16, tag="vh")
                    nc.sync.dma_start(
                        vh[:],
                        v_d[b][:, :].rearrange("(so p) d -> p so d", p=P)
                    )
                for h in range(NHEAD):
                    if b == 0 and h < 2:
                        qh = prefetch["qh"][h]
                    else:
                        qh = qpool.tile([P, S], F16, tag="qh")
                        nc.gpsimd.dma_start(qh[:],
                                            qT_d[b][h * P:(h + 1) * P, :])
                    psden = ps_den.tile([4, TT2], F32, tag="psden")
                    aouns = []

                    for t_idx in range(NT2):
                        if t_idx == 1:
                            flush(0)    # prev (b,h): reciprocal halves
                        t0 = t_idx * TT2
                        n_s = 4 * (t_idx + 1)   # visible 128-wide s-tiles
                        psO = ps_o.tile([P, TT2], F32, tag="psO")
                        for st in range(n_s):
                            if st == 4:
                                if t_idx == 2:
                                    flush(1)    # prev: normalize t0,t1
                                elif t_idx == 3:
                                    flush(2)    # prev: normalize t2,t3
                            pssc = ps_sc.tile([P, TT2], F32, tag="pssc")
                            nc.tensor.matmul(
                                pssc[:], kh[:, st * P:(st + 1) * P],
                                qh[:, t0:t0 + TT2], start=True, stop=True,
                            )
                            esb = epool.tile([P, TT2], BF16, tag="esb")
                            dg = st - 4 * t_idx
                            if dg < 0:          # fully visible s-tile
                                nc.scalar.activation(
                                    esb[:], pssc[:],
                                    mybir.ActivationFunctionType.Exp,
                                    scale=SCALE, bias=bias_sb[:, 0:1],
                                )
                            else:               # diagonal s-tile
                                c0 = P * dg
                                nc.vector.tensor_add(
                                    pssc[:, c0:c0 + P],
                                    pssc[:, c0:c0 + P], tri_sb[:],
                                )
                                if dg > 0:
                                    nc.vector.memset(esb[:, 0:c0], 0.0)
                                nc.scalar.activation(
                                    esb[:, c0:TT2], pssc[:, c0:TT2],
                                    mybir.ActivationFunctionType.Exp,
                                    scale=SCALE, bias=bias_sb[:, 0:1],
                                )
                            z0 = P * dg if dg > 0 else 0
                            nc.tensor.matmul(
                                psO[:, z0:TT2], vh[:, st, :], esb[:, z0:TT2],
                                start=(st == 0), stop=(st == n_s - 1),
                            )
                            nc.tensor.matmul(
                                psden[:, z0:TT2],
                                colC4_sb[:, 4 * t_idx:4 * t_idx + 4],
                                esb[:, z0:TT2],
                                start=(t_idx == 0 and st == 0),
                                stop=(t_idx == NT2 - 1 and st == n_s - 1),
                            )
                        # /256 keeps the unnormalized sum inside f16 range;
                        # recip carries the matching 256/sum(e) factor.
                        aoun = opool.tile([P, TT2], F16, tag="aoun", bufs=8)
                        nc.vector.tensor_scalar_mul(aoun[:], psO[:], 1.0 / 256.0)
                        aouns.append(aoun)

                    # deferred tail for this (b,h): recip + normalize + store
                    recip4 = opool.tile([4, TT2], F32, tag="recip4")
                    rflat = opool.tile([1, 4, TT2], F32, tag="rflat",
                                       bufs=1)

                    def mk_recip(psden=psden, recip4=recip4, rflat=rflat):
                        def go():
                            nc.vector.reciprocal(recip4[:, 0:TT2 // 2],
                                                 psden[:, 0:TT2 // 2])
                            nc.vector.reciprocal(recip4[:, TT2 // 2:],
                                                 psden[:, TT2 // 2:])
                            nc.gpsimd.dma_start(rflat[:], recip4[:])
                        return go

                    def mk_norm(t_idx, b=b, h=h, rflat=rflat, aouns=aouns):
                        def go():
                            psbc = ps_sc.tile([P, TT2], F32, tag="pssc",
                                              name="psbc")
                            nc.tensor.matmul(
                                psbc[:], ones32[:],
                                rflat[:, t_idx, :], start=True, stop=True,
                            )
                            osb = opool.tile([P, TT2], F16, tag="osb", bufs=3)
                            nc.vector.tensor_mul(osb[:], aouns[t_idx][:],
                                                 psbc[:])
                            nc.sync.dma_start(
                                ao_d[b][h * P:(h + 1) * P,
                                        t_idx * TT2:(t_idx + 1) * TT2],
                                osb[:],
                            )
                        return go

                    pending.append((0, mk_recip()))
                    pending.append((1, mk_norm(0)))
                    pending.append((1, mk_norm(1)))
                    pending.append((2, mk_norm(2)))
                    pending.append((2, mk_norm(3)))

            for st, fn in pending:      # last (b,h)'s tail
                fn()
            pending.clear()

        qpool.release()
        kvpool.release()
        # ---------------- Phase 3: wo partial projection -------------------
        with (
            tc.tile_pool(name="p3a", bufs=3) as apool,
            tc.tile_pool(name="p3o", bufs=4) as opool3,
            tc.tile_pool(name="p3ps", bufs=4, space="PSUM") as pspool3,
        ):
            for ti in range(T // TT3):
                b = ti // (S // TT3)
                lt0 = (ti % (S // TT3)) * TT3
                t0 = ti * TT3
                ao_sb = apool.tile([P, QPC // P, TT3], F16, tag="ao")
                nc.gpsimd.dma_start(
                    ao_sb[:],
                    ao_d[b][:, lt0:lt0 + TT3].rearrange(
                        "(jo p) t -> p jo t", p=P),
                )
                for half in range(2):
                    psws = []
                    for ii_l in range(4):
                        psw = pspool3.tile([P, IT3], F32, tag="psw",
                                           name=f"psw{ii_l}")
                        psws.append(psw)
                    for jo in range(QPC // P):
                        for ii_l in range(4):
                            ii = half * 4 + ii_l
                            nc.tensor.matmul(
                                psws[ii_l][:], ao_sb[:, jo, :],
                                wo_sb[:, jo, ii * IT3:(ii + 1) * IT3],
                                start=(jo == 0), stop=(jo == QPC // P - 1),
                            )
                    for ii_l in range(4):
                        ii = half * 4 + ii_l
                        ow = opool3.tile([P, IT3], F16, tag="ow")
                        if ii_l % 2 == 0:
                            nc.vector.tensor_copy(ow[:], psws[ii_l][:])
                        else:
                            nc.scalar.copy(ow[:], psws[ii_l][:])
                        nc.sync.dma_start(
                            out[t0:t0 + TT3, ii * IT3:(ii + 1) * IT3], ow[:]
                        )
        wpool3.release()

    return nc


_NC_CACHE = {}


def _host_inputs(x, wq, wk, wv, wo, freqs_cos, freqs_sin):
    f16 = np.float16
    xT = np.ascontiguousarray(x.reshape(T, DIM).T, dtype=f16)
    C = np.ascontiguousarray(np.repeat(freqs_cos.T, 2, axis=0), dtype=f16)
    sign = np.where(np.arange(HD) % 2 == 0, -1.0, 1.0)[:, None].astype(np.float32)
    Sp = np.ascontiguousarray(np.repeat(freqs_sin.T, 2, axis=0) * sign,
                              dtype=f16)
    perm = np.zeros((P, P), f16)
    idx = np.arange(0, P, 2)
    perm[idx, idx + 1] = 1.0
    perm[idx + 1, idx] = 1.0
    ident = np.eye(P, dtype=f16)
    colC4 = np.zeros((P, 4 * NT2), np.float32)
    for t in range(NT2):
        colC4[:, 4 * t + t] = 1.0 / 256.0
    colC4 = colC4.astype(ml_dtypes.bfloat16)
    tri = np.where(np.arange(P)[None, :] >= np.arange(P)[:, None],
                   0.0, -1e9).astype(np.float32)
    biasv = np.full((P, 1), -4.0, np.float32)

    in_maps = []
    for c in range(N_CORES):
        in_maps.append({
            "xT": xT,
            "wqT": np.ascontiguousarray(wq[c * QPC:(c + 1) * QPC, :].T, dtype=f16),
            "wkT": np.ascontiguousarray(wk[c * HD:(c + 1) * HD, :].T, dtype=f16),
            "wvT": np.ascontiguousarray(wv[c * HD:(c + 1) * HD, :].T, dtype=f16),
            "woT": np.ascontiguousarray(wo[:, c * QPC:(c + 1) * QPC].T, dtype=f16),
            "ropeC": C,
            "ropeS": Sp,
            "permM": perm,
            "identM": ident,
            "colC4M": colC4,
            "triM": tri,
            "biasM": biasv,
        })
    return in_maps


def kernel(x, wq, wk, wv, wo, freqs_cos, freqs_sin, start_pos, _trace=False):
    x = np.asarray(x, np.float32)
    in_maps = _host_inputs(
        x, np.asarray(wq, np.float32), np.asarray(wk, np.float32),
        np.asarray(wv, np.float32), np.asarray(wo, np.float32),
        np.asarray(freqs_cos, np.float32), np.asarray(freqs_sin, np.float32),
    )
    if "nc" not in _NC_CACHE:
        nc = build_kernel()
        nc.compile()
        _NC_CACHE["nc"] = nc
    res = run_bass_kernel_spmd(
        _NC_CACHE["nc"], in_maps, list(range(N_CORES)), trace=_trace
    )
    acc = res.results[0]["out_part"].astype(np.float32)
    for c in range(1, N_CORES):
        acc += res.results[c]["out_part"].astype(np.float32)
    out = acc.reshape(B, S, DIM)
    if _trace:
        return out, res
    return out
